# revision 44
# baseline (speedup 1.0000x reference)
"""Trainium2 Bass kernel for nn_Damping (B=32768, N=64, H=256).

Per-sample computation:
    diag = (relu(MLP_d(x)) + damp_min) * x          # [64]
    off  = MLP_o(x)                                  # [2016] strictly-lower entries
    L    = scatter(off -> strict lower, diag -> diagonal)   # [64, 64]
    out  = L @ (L^T @ x)

Strategy: pure data parallel over 8 NeuronCores (4096 samples each).
On-chip layout is feature-major: x arrives pre-transposed as bf16
[128, 4096] (bottom half duplicates the top) and the output leaves as
f32 [128, 4096] whose partition halves the host sums and transposes.

Scatter matvecs avoid materializing L:
    v   = Ecol^T (off * (Rrow @ xT)) + diag * x       (v = L^T x)
    out = Erow^T (off * (Rcol @ vT)) + diag * v       (out = L v)
with PE-array packing (tile_position) exploited throughout:
  - the Ecol/Erow reductions of a slice pair run CONCURRENTLY as two
    col-tiles (M=64) accumulating into the top/bottom partition halves
    of one PSUM bank (v and out are kept as un-folded half sums),
  - pass-2 expansion stationaries are stacked [Rcol_s; Rcol_s] so the
    K=128 contraction folds the v halves for free,
  - diag*x / diag*v fold into the same accumulators via small K=64
    identity matmuls (the whole diag path is computed on 128
    partitions with duplicated content, so no cross-partition adds
    are ever needed; the host sums the two output halves),
  - MLP L1 (K=64) runs as two concurrent row-tiles on duplicated x.
Emission is software-pipelined so the PE queue never head-of-line
blocks on DVE/Act results (reductions trail their inputs by 2 pairs;
next block's MLP matmuls fill the PE while v is assembled).
"""

import numpy as np

B, N, H, OFF = 32768, 64, 256, 2016
NCORES = 8
BLOCAL = B // NCORES          # 4096 samples per core
NSLICES = 16
SL = 128                      # padded slice width; 16*128 = 2048
OFFP = NSLICES * SL           # 2048 (padded off dim)
NBLOCKS = 8                   # blocks of 512 samples per core
BT = 512                      # batch tile (moving free dim)
NPAIRS = NSLICES // 2         # slice pairs
NWARM = 14                    # PE warmup matmuls (HAM un-throttle)

_compiled = {}


def _build_program(with_boo=True):
    import concourse.bass as bass  # noqa: F401
    import concourse.mybir as mybir
    import concourse.tile as tile
    from concourse import bacc

    f32 = mybir.dt.float32
    bf16 = mybir.dt.bfloat16
    AF = mybir.ActivationFunctionType

    nc = bacc.Bacc("TRN2", target_bir_lowering=False, debug=False,
                   num_devices=NCORES)

    def din(name, shape, dt=f32):
        return nc.dram_tensor(name, list(shape), dt, kind="ExternalInput").ap()

    xt_ap = din("xt", (128, BLOCAL), bf16)     # bottom 64 partitions = dup(x)
    xe1_ap = din("xe1", (SL, NSLICES, BLOCAL), bf16)
    # all small weights in ONE dram tensor -> one DMA trigger, lands early:
    # cols 0:128 idt | 128:256 wd1 | 256:768 wd2 | 768:1024 wdo |
    # 1024:1152 wo1 | 1152:1664 wo2 | 1664:2176 dmf
    wpack_ap = din("wpack", (128, 2176), bf16)
    woo_ap = din("woo", (128, 2, OFFP), bf16)
    # biases: cols 0-1 bd1, 2-3 bo1, 4-5 bd2, 6-7 bo2, 8 bdo (dup'd rows)
    blob_ap = din("blob", (128, 9))
    ecol_ap = din("ecol", (SL, NSLICES * 64), bf16)   # M=64 reduction stats
    erow_ap = din("erow", (SL, NSLICES * 64), bf16)
    rcol_ap = din("rcol", (128, OFFP), bf16)   # stacked [Rcol; Rcol]
    if with_boo:
        blobb_ap = din("blobb", (128, 192), bf16)  # b1 [128,128], b2 [128,64]
    out_ap = nc.dram_tensor("out", [128, BLOCAL], f32,
                            kind="ExternalOutput").ap()

    with tile.TileContext(nc) as tc:
        with (
            tc.tile_pool(name="consts", bufs=1) as consts,
            tc.tile_pool(name="acts", bufs=2) as act_pool,
            tc.tile_pool(name="acts3", bufs=3) as act3_pool,
            tc.tile_pool(name="offp", bufs=2) as off_pool,
            tc.tile_pool(name="mp", bufs=4) as m_pool,
            tc.tile_pool(name="small", bufs=2) as small_pool,
            tc.tile_pool(name="outp", bufs=2) as out_pool,
            tc.tile_pool(name="vdp", bufs=2) as vd_pool,
            tc.tile_pool(name="xe1", bufs=2) as xe_pool,
            # PSUM: 8 banks of [128, 512] f32 total.
            tc.tile_pool(name="ps_a", bufs=2, space="PSUM") as ps_a,      # 2
            tc.tile_pool(name="ps_big", bufs=2, space="PSUM") as ps_big,  # 4
            tc.tile_pool(name="ps_acc", bufs=2, space="PSUM") as ps_acc,  # 2
        ):
            # ---- load constants (ordered by need time) ----
            # DMA trigger instructions occupy their issuing engine's queue,
            # so the scalar (Act) engine issues NONE; loads spread across
            # sync / gpsimd / vector rings by when the data is needed.
            def load(name, shape, ap, eng):
                t = consts.tile(list(shape), ap.dtype, tag=name, name=name)
                eng.dma_start(t[:], ap)
                return t

            xts = [None] * NBLOCKS

            def load_xt(b, eng):
                t = consts.tile([128, BT], bf16, tag=f"xt{b}", name=f"xt{b}")
                eng.dma_start(t[:], xt_ap[:, BT * b:BT * (b + 1)])
                xts[b] = t

            # double-buffered pass-1 expansion tiles; per-pair DMA triggers
            # so arrival rate tracks consumption (never floods HBM)
            xe_tiles = [None] * NBLOCKS

            def prefetch_xe(b, eng=None, frm=0, upto=NPAIRS):
                if b < NBLOCKS:
                    if frm == 0:
                        xe_tiles[b] = xe_pool.tile([SL, NSLICES, BT], bf16,
                                                   tag="xe1", name="xe1")
                    t = xe_tiles[b]
                    for q in range(frm, upto):
                        (eng or nc.sync).dma_start(
                            t[:, 2 * q:2 * q + 2],
                            xe1_ap[:, 2 * q:2 * q + 2, BT * b:BT * (b + 1)])

            # Startup is HBM-bound and the 16 hw DMA queues fair-share
            # bandwidth, so priority = explicit dependency gates: the sync
            # ring carries only the critical path (weights then woo); all
            # scatter-phase data loads sit on the gpsimd ring BEHIND a tiny
            # compute op that reads the woo tile, so they only start once
            # woo has landed.
            wpack = load("wpack", (128, 2176), wpack_ap, nc.sync)
            blob = load("blob", (128, 9), blob_ap, nc.sync)
            load_xt(0, nc.sync)
            load_xt(1, nc.sync)
            load_xt(2, nc.sync)
            if with_boo:
                blobb = load("blobb", (128, 192), blobb_ap, nc.sync)
            woo = load("woo", (128, 2, OFFP), woo_ap, nc.sync)
            # force the gpsimd elementwise ucode lib load (~6us) NOW, off
            # the critical path, by issuing a tiny op on the first-landing
            # const; the real diag-path ops then start without the stall
            _libw = small_pool.tile([1, 4], f32, tag="libw")
            nc.gpsimd.tensor_add(out=_libw[:], in0=blob[0:1, 0:4],
                                 in1=blob[0:1, 0:4])
            nc.gpsimd.tensor_add(out=_libw[:], in0=woo[0:1, 0, 0:4],
                                 in1=woo[0:1, 0, 0:4])
            prefetch_xe(0, nc.gpsimd, upto=3)
            ecol = load("ecol", (SL, NSLICES * 64), ecol_ap, nc.gpsimd)
            erow = load("erow", (SL, NSLICES * 64), erow_ap, nc.gpsimd)
            prefetch_xe(0, nc.gpsimd, frm=3)
            rcol = load("rcol", (128, OFFP), rcol_ap, nc.gpsimd)
            # x tiles for blocks 3+ are needed only ~60us in; keeping them
            # off the early sync ring stops them starving wpack/woo (the hw
            # DMA subqueues fair-share HBM regardless of issue order)
            for _b in range(3, NBLOCKS):
                load_xt(_b, nc.gpsimd)
            bd1, bo1 = blob[:, 0:2], blob[:, 2:4]
            bd2, bo2 = blob[:, 4:6], blob[:, 6:8]
            bdo = blob[:, 8:9]
            idt = wpack[:, 0:128]
            wd1 = wpack[:, 128:256]
            wo1 = wpack[:, 1024:1152]
            dmf = wpack[:, 1664:2176]

            def wd2_sl(k, s):
                o = 256 + 256 * k + 128 * s
                return wpack[:, o:o + 128]

            def wo2_sl(k, s):
                o = 1152 + 256 * k + 128 * s
                return wpack[:, o:o + 128]

            def wdo_sl(k):
                o = 768 + 128 * k
                return wpack[:, o:o + 128]

            id_a = wpack[0:64, 0:64]          # identity on partitions 0-63
            id_b = wpack[64:128, 64:128]      # identity on partitions 64-127
            if with_boo:
                b1 = blobb[:, 0:128]
                b2 = blobb[:, 128:192]        # stacked [b2; b2], M=64

            def keep_warm(n):
                """No-dependency matmuls that hold PE activity up through
                short DVE-wait gaps so HAM never re-throttles the clock."""
                for _w in range(n):
                    wps = ps_a.tile([128, BT], f32, tag="mlp")
                    nc.tensor.matmul(wps[:, 0:128], idt, idt, start=True,
                                     stop=True)

            # ---- PE warmup: un-throttle HAM while weights stream in ----
            keep_warm(NWARM)

            def mlp_l1(w1, b1v, xtile, tag):
                """L1: two concurrent K=64 row-tiles (x is duplicated into
                the bottom partitions) into the two banks of one PSUM pair
                tile, then per-slice tanh."""
                a1 = act_pool.tile([128, 2, BT], bf16, tag=tag)
                ps1 = ps_big.tile([128, 2, BT], f32, tag="big")
                nc.tensor.matmul(ps1[:, 0], w1[0:64, :], xtile[0:64, :],
                                 start=True, stop=True)
                nc.tensor.matmul(ps1[:, 1], w1[64:128, :], xtile[64:128, :],
                                 start=True, stop=True)
                for s in range(2):
                    nc.scalar.activation(a1[:, s], ps1[:, s], AF.Tanh,
                                         bias=b1v[:, s:s + 1])
                return a1

            def mlp_l2(w2sl, b2v, a1, tag):
                a2 = act3_pool.tile([128, 2, BT], bf16, tag=tag)
                ps2 = ps_big.tile([128, 2, BT], f32, tag="big")
                for s in range(2):
                    for k in range(2):
                        nc.tensor.matmul(ps2[:, s], w2sl(k, s),
                                         a1[:, k], start=(k == 0),
                                         stop=(k == 1))
                    nc.scalar.activation(a2[:, s], ps2[:, s], AF.Tanh,
                                         bias=b2v[:, s:s + 1])
                return a2

            def scatter_pass1(off, xe, g2, psv, dvx, xtile, pending=None,
                              mid=None):
                """psv halves accumulate Ecol^T (off * xe) (+ b1 x + diag*x).

                off = Woo@g2; the pass-1 expansion xe is precomputed on the
                host and streamed from HBM, so the multiply is all-SBUF bf16
                (2x DVE). Reduction col-tile pairs for pair q are emitted
                inside iteration q+2 so the PE never waits on the DVE; the
                two tiles of a pair run concurrently on the two column
                halves of the PE array."""
                if with_boo:
                    nc.tensor.matmul(psv[:], b1, xtile,
                                     start=True, stop=False)
                m1s = [None] * NPAIRS

                def emit_red(q, last=False):
                    st = (not with_boo) and q == 0
                    nc.tensor.matmul(psv[0:64], ecol[:, 64 * 2 * q:
                                                     64 * (2 * q + 1)],
                                     m1s[q][:, 0], start=st, stop=False)
                    nc.tensor.matmul(psv[64:128], ecol[:, 64 * (2 * q + 1):
                                                       64 * (2 * q + 2)],
                                     m1s[q][:, 1], start=st, stop=last)

                for q in range(NPAIRS):
                    if q == 1 and pending is not None:
                        pending()
                    if q == 6 and mid is not None:
                        mid()   # next-next block's L1: its tanhs precede
                        # the last off-copies in the Act FIFO, so the L2
                        # matmuls never stall on them
                    pso = ps_big.tile([128, 2, BT], f32, tag="big")
                    for j in range(2):
                        s = 2 * q + j
                        for k in range(2):
                            nc.tensor.matmul(
                                pso[:, j],
                                woo[:, k, SL * s:SL * (s + 1)],
                                g2[:, k], start=(k == 0), stop=(k == 1))
                    nc.scalar.copy(off[:, 2 * q:2 * q + 2], pso[:])
                    if q > 1:
                        emit_red(q - 2)
                    m1 = m_pool.tile([128, 2, BT], bf16, tag="m1")
                    m1s[q] = m1
                    nc.vector.tensor_mul(out=m1[:], in0=off[:, 2 * q:2 * q + 2],
                                         in1=xe[:, 2 * q:2 * q + 2])

                def finish():
                    emit_red(NPAIRS - 2)
                    emit_red(NPAIRS - 1, last=True)
                    # diag*x folds into the top half (stop for partitions
                    # 0-63); the bottom half stopped at the last reduction
                    nc.tensor.matmul(psv[0:64], id_a, dvx[0:64, :],
                                     start=False, stop=True)
                return finish

            def scatter_pass2(b, off, vd, pso2, diag):
                """pso2 halves accumulate Erow^T (off * (Rcol vd))
                (+ b2 v + diag*v); the host sums the halves."""
                if with_boo:
                    nc.tensor.matmul(pso2[0:64], b2, vd[:],
                                     start=True, stop=False)
                dvv = small_pool.tile([128, BT], bf16, tag="dvv")
                nc.gpsimd.tensor_mul(out=dvv[:], in0=diag[:], in1=vd[:])
                m2s = [None] * NPAIRS

                def emit_red(q, first):
                    st = (not with_boo) and first
                    nc.tensor.matmul(pso2[0:64], erow[:, 64 * 2 * q:
                                                      64 * (2 * q + 1)],
                                     m2s[q][:, 0], start=st, stop=False)
                    nc.tensor.matmul(pso2[64:128], erow[:, 64 * (2 * q + 1):
                                                        64 * (2 * q + 2)],
                                     m2s[q][:, 1], start=first, stop=False)

                # pair processing order is rotated so the last two ps_big
                # acquisitions of this pass have long-completed readers by
                # the time the next block's woo matmuls recycle the pool
                order = [(i + NPAIRS - 2) % NPAIRS for i in range(NPAIRS)]
                for i, q in enumerate(order):
                    pse = ps_big.tile([128, 2, BT], f32, tag="big")
                    for j in range(2):
                        s = 2 * q + j
                        nc.tensor.matmul(
                            pse[:, j],
                            rcol[:, SL * s:SL * (s + 1)],
                            vd[:], start=True, stop=True)
                    if i > 1:
                        if b >= NBLOCKS - 2:
                            keep_warm(4)
                        emit_red(order[i - 2], first=(i == 2))
                    m2 = m_pool.tile([128, 2, BT], bf16, tag="m2")
                    m2s[q] = m2
                    if ((b == NBLOCKS - 1 and i >= 3)
                            or (b == NBLOCKS - 2 and i >= 6)):
                        # tail rebalance: Act converts PSUM->SBUF so the
                        # last multiplies run in 2x all-SBUF DVE mode
                        pse_sb = m_pool.tile([128, 2, BT], bf16, tag="psb")
                        nc.scalar.copy(pse_sb[:], pse[:])
                        nc.vector.tensor_mul(out=m2[:],
                                             in0=off[:, 2 * q:2 * q + 2],
                                             in1=pse_sb[:])
                    else:
                        nc.vector.tensor_mul(out=m2[:],
                                             in0=off[:, 2 * q:2 * q + 2],
                                             in1=pse[:])

                def finish():
                    if b >= NBLOCKS - 2:
                        keep_warm(4)
                    emit_red(order[NPAIRS - 2], first=False)
                    if b >= NBLOCKS - 2:
                        keep_warm(4)
                    emit_red(order[NPAIRS - 1], first=False)
                    # diag*v folds in as a concurrent col-tile pair
                    nc.tensor.matmul(pso2[0:64], id_a, dvv[0:64, :],
                                     start=False, stop=True)
                    nc.tensor.matmul(pso2[64:128], id_b, dvv[64:128, :],
                                     start=False, stop=True)
                    outf = out_pool.tile([128, BT], f32, tag="outf",
                                         name="outf")
                    nc.scalar.copy(outf[:], pso2[:])
                    nc.sync.dma_start(out_ap[:, BT * b:BT * (b + 1)],
                                      outf[:])
                return finish

            def mlp_start(b):
                return mlp_l1(wd1, bd1, xts[b], "h1")

            def mlp_rest(b, a1h, pending=None):
                """Remainder of block b's MLPs. `pending` (deferred tail
                reductions of the previous pass) is emitted between the
                MLPs so those matmuls never head-of-line block the PE
                queue while their DVE inputs finish."""
                h2 = mlp_l2(wd2_sl, bd2, a1h, "h2")
                if pending is not None:
                    pending()
                a1g = mlp_l1(wo1, bo1, xts[b], "g1")
                g2 = mlp_l2(wo2_sl, bo2, a1g, "g2")
                return h2, g2

            # MLPs run TWO blocks ahead: they need only the small weights,
            # so the PE stays busy during the HBM-bound xe/woo load phase.
            mlps_cur = mlp_rest(0, mlp_start(0))
            mlps_nxt = mlp_rest(1, mlp_start(1))
            a1h_nn = [None]     # L1-h of block b+2, emitted mid-pass-1
            fin2 = None
            for b in range(NBLOCKS):
                xtile = xts[b]
                h2, g2 = mlps_cur

                # ---- diag = (relu(d + bdo) + dm) * x on 128 duplicated
                # partitions (wdo columns are duplicated) ----
                psd = ps_a.tile([128, BT], f32, tag="mlp")
                for k in range(2):
                    nc.tensor.matmul(psd[:], wdo_sl(k), h2[:, k],
                                     start=(k == 0), stop=(k == 1))
                dr = small_pool.tile([128, BT], bf16, tag="dr")
                nc.scalar.activation(dr[:], psd[:], AF.Relu, bias=bdo)
                dd = small_pool.tile([128, BT], bf16, tag="dd")
                nc.gpsimd.tensor_add(out=dd[:], in0=dr[:], in1=dmf[:])
                diag = small_pool.tile([128, BT], bf16, tag="diag")
                nc.gpsimd.tensor_mul(out=diag[:], in0=dd[:], in1=xtile[:])
                dvx = small_pool.tile([128, BT], bf16, tag="dvx")
                nc.gpsimd.tensor_mul(out=dvx[:], in0=diag[:], in1=xtile[:])

                # ---- pass 1 ----
                off = off_pool.tile([SL, NSLICES, BT], bf16, tag="off")
                psv = ps_acc.tile([128, BT], f32, tag="acc")
                prefetch_xe(b + 1)
                mid = None
                if b + 2 < NBLOCKS:
                    def mid(bb=b + 2):
                        a1h_nn[0] = mlp_start(bb)
                fin1 = scatter_pass1(off, xe_tiles[b], g2, psv, dvx,
                                     xtile[:], pending=fin2, mid=mid)

                # block b+2's MLP matmuls fill the PE while v is assembled
                if b + 2 < NBLOCKS:
                    mlps_cur = mlps_nxt
                    mlps_nxt = mlp_rest(b + 2, a1h_nn[0], pending=fin1)
                elif b + 1 < NBLOCKS:
                    mlps_cur = mlps_nxt
                    fin1()
                else:
                    fin1()
                # v (as un-folded halves) -> SBUF bf16; pass-2 expansion
                # stationaries are stacked so the halves sum inside the PE
                vd = vd_pool.tile([128, BT], bf16, tag="vd")
                nc.scalar.copy(vd[:], psv[:])

                # ---- pass 2 ----
                pso2 = ps_acc.tile([128, BT], f32, tag="acc")
                fin2t = scatter_pass2(b, off, vd, pso2, diag)

                if b == NBLOCKS - 1:
                    fin2t()
                else:
                    fin2 = fin2t

    nc.compile()
    return nc


def _get_program(with_boo=True):
    if with_boo not in _compiled:
        _compiled[with_boo] = _build_program(with_boo)
    return _compiled[with_boo]


def _host_consts(inputs):
    import ml_dtypes
    f = np.float32
    bf = ml_dtypes.bfloat16
    rows, cols = np.tril_indices(N, k=-1)         # length 2016

    ecol = np.zeros((SL, NSLICES, 64), f)
    erow = np.zeros((SL, NSLICES, 64), f)
    for s in range(NSLICES):
        for m in range(SL):
            p = SL * s + m
            if p < len(rows):
                ecol[m, s, cols[p]] = 1.0
                erow[m, s, rows[p]] = 1.0

    # stacked expansion: rows 0-63 and 64-127 both map v-col c -> positions
    rcol = np.zeros((128, OFFP), f)
    for p in range(len(rows)):
        rcol[cols[p], p] = 1.0
        rcol[64 + cols[p], p] = 1.0

    woo_pad = np.zeros((H, OFFP), f)
    woo_pad[:, :OFF] = np.asarray(inputs["Woo"], f)

    boo_v = np.asarray(inputs["boo"], f)
    blobb = np.zeros((128, 192), f)
    # b1: v_c += boo_rc * x_r   (K = x rows 0-63, M = 128 w/ cols 64+ zero)
    blobb[rows, cols] = boo_v
    # b2 stacked: out_r += boo_rc * v_c, fed with vd (v in halves)
    blobb[cols, 128 + rows] = boo_v
    blobb[64 + cols, 128 + rows] = boo_v

    def bt2(v):  # [256] -> [128, 2]
        return np.asarray(v, f).reshape(2, 128).T

    blob = np.zeros((128, 9), f)
    blob[:, 0:2] = bt2(inputs["bd1"])
    blob[:, 2:4] = bt2(inputs["bo1"])
    blob[:, 4:6] = bt2(inputs["bd2"])
    blob[:, 6:8] = bt2(inputs["bo2"])
    bdo_d = np.asarray(inputs["bdo"], f)
    blob[:N, 8] = bdo_d
    blob[N:, 8] = bdo_d

    def l1t(w):  # [64, 256] -> [128, 128] row-tiled halves
        w = np.asarray(w, f)
        out = np.zeros((128, 128), f)
        out[0:64] = w[:, 0:128]
        out[64:128] = w[:, 128:256]
        return out

    def kt(w):  # [256, M] -> [128, 2, M]
        w = np.asarray(w, f)
        return np.ascontiguousarray(w.reshape(2, 128, -1).transpose(1, 0, 2))

    wdo_d = np.asarray(inputs["Wdo"], f)                   # [256, 64]
    wdo_dup = np.concatenate([wdo_d, wdo_d], axis=1)       # duplicate cols

    wpack = np.zeros((128, 2176), f)
    wpack[:, 0:128] = np.eye(128, dtype=f)
    wpack[:, 128:256] = l1t(inputs["Wd1"])
    wpack[:, 256:768] = kt(inputs["Wd2"]).reshape(128, 512)
    wpack[:, 768:1024] = kt(wdo_dup).reshape(128, 256)
    wpack[:, 1024:1152] = l1t(inputs["Wo1"])
    wpack[:, 1152:1664] = kt(inputs["Wo2"]).reshape(128, 512)
    wpack[:N, 1664:2176] = np.asarray(
        inputs["damp_min"], f).reshape(N, 1)
    wpack[N:, 1664:2176] = wpack[:N, 1664:2176]

    return {
        "wpack": wpack.astype(bf),
        "woo": kt(woo_pad).astype(bf),
        "blob": blob,
        "blobb": blobb.astype(bf),
        "ecol": np.ascontiguousarray(
            ecol.reshape(SL, NSLICES * 64)).astype(bf),
        "erow": np.ascontiguousarray(
            erow.reshape(SL, NSLICES * 64)).astype(bf),
        "rcol": rcol.astype(bf),
    }


def kernel(trace=False, **inputs):
    import ml_dtypes
    from concourse.bass_utils import run_bass_kernel_spmd

    with_boo = bool(np.any(np.asarray(inputs["boo"])))
    nc = _get_program(with_boo=with_boo)
    consts = _host_consts(inputs)
    if not with_boo:
        consts.pop("blobb")
    xt = np.asarray(inputs["x"], np.float32).T.astype(ml_dtypes.bfloat16)
    rows, _ = np.tril_indices(N, k=-1)
    rows_p = np.concatenate([rows, np.zeros(OFFP - len(rows), int)])
    in_maps = []
    for i in range(NCORES):
        xt_c = np.empty((128, BLOCAL), ml_dtypes.bfloat16)
        xt_c[:N] = xt[:, i * BLOCAL:(i + 1) * BLOCAL]
        xt_c[N:] = xt_c[:N]
        xe1_c = np.ascontiguousarray(
            xt_c[rows_p].reshape(NSLICES, SL, BLOCAL).transpose(1, 0, 2))
        in_maps.append({"xt": xt_c, "xe1": xe1_c, **consts})
    res = run_bass_kernel_spmd(nc, in_maps, core_ids=list(range(NCORES)),
                               trace=trace)
    out = np.concatenate(
        [np.ascontiguousarray(
            (res.results[i]["out"][:N] + res.results[i]["out"][N:]).T)
         for i in range(NCORES)],
        axis=0)
    if trace:
        kernel.last_results = res
    return out
